# revision 1
# baseline (speedup 1.0000x reference)
"""Trainium2 Bass kernel for the Dirichlet-KDE ECE loss (nn_KDEECE).

reference math (N=8192, C=10, h=0.1):
  f        = softmax(logits)                      [N, C]
  alphas   = f/h + 1                              (sum_c alphas == C + 1/h == 20)
  log_beta = sum_c lgamma(alphas) - lgamma(20)    [N]
  log_kern = log(f+eps) @ (10 f).T - log_beta[j], diag = -inf
  kern     = exp(log_kern)
  out      = mean_i sum_c | (kern @ onehot)/rowsum(kern) - f |

Device strategy (8 cores, rows i sharded, 1024 per core):
  Layout: j on SBUF partitions, i on the free axis.  Fold -log_beta into the
  matmul as an 11th contraction term, so  log_kern.T = A'^T.T @ L'^T  with
  A' = [10 f | -log_beta]  and  L' = [log(f+eps) | 1].  Per 128-wide j tile:
    PE : psum_ln[128 j, 1024 i]  = matmul(lhsT=A'_T[:, jt], rhs=L'_T_loc)
    ACT: kern = exp(psum_ln)                       (ScalarE, fused)
    PE : psum_ky[11, 1024 i]    += matmul(lhsT=[onehot | 1][jt], rhs=kern)
  psum_ky rows 0..9 are kern @ onehot (transposed), row 10 is rowsum(kern).
  The leave-one-out diagonal is not masked on-device; the host subtracts the
  analytically known self-term kern_ii afterwards (validated: no cancellation,
  rel err ~1e-6).  Host work is O(N*C) only.
"""

import os

import numpy as np

N, C = 8192, 10
H_INV = 10.0
EPS_LOG = 1e-45
EPS_DEN = 1e-10
N_CORES = 8
LOC = N // N_CORES  # 1024 rows of i per core
JT = N // 128  # 64 j tiles
CK = C + 1  # 10 classes + rowsum column

_compiled = None  # (nc, tensor names) cache across calls

# perf variants (validated empirically against the reference on HW):
#  LN_F32R : run the [11]-contraction log_num matmul in float32r (1 cyc/row
#            instead of fp32's 4)
#  KY_BF16 : ACT writes kern tiles as bf16; the onehot accumulation matmul
#            then also runs at 1 cyc/row
LN_F32R = True
KY_BF16 = True
KERN_BUFS = 6  # SBUF kern-tile double buffering depth
WARMUP = 0  # dummy PE matmuls before the loop (p-state ramp)
KY_FUSED = False  # single 1024-wide bf16 onehot matmul per j-tile
PIPE_DEPTH = 3  # iterations between ln(jt) emission and ky(jt) emission
AP_DMA_CHUNKS = 4  # split the apT load so ln(0) starts early (unused)
DVE_RELEASE = False  # add a tiny DVE read of each ln tile so the pool
# release bookkeeping lands on idle DVE instead of busy ACT
DIRECT_OUT = False  # DMA the ky accumulator straight from PSUM to DRAM


def _lgamma(x):
    try:
        from scipy.special import gammaln

        return gammaln(x)
    except Exception:
        import math

        return np.vectorize(math.lgamma)(x.astype(np.float64))


def _host_prep(logits, labels):
    logits = np.asarray(logits, np.float32)
    labels = np.asarray(labels).astype(np.int64)
    x = logits - logits.max(axis=1, keepdims=True)
    e = np.exp(x)
    f = (e / e.sum(axis=1, keepdims=True)).astype(np.float32)

    alphas = (f.astype(np.float64) * H_INV) + 1.0
    log_beta = (_lgamma(alphas).sum(axis=1) - _lgamma(np.full(N, C + H_INV))).astype(
        np.float32
    )
    L = np.log(f + EPS_LOG).astype(np.float32)
    A = (H_INV * f).astype(np.float32)

    # A' = [A | -log_beta]  -> transposed [11, N]; L' = [L | 1] -> [11, N]
    apT = np.concatenate([A, -log_beta[:, None]], axis=1).T.copy()  # [11, N]
    lpT = np.concatenate([L, np.ones((N, 1), np.float32)], axis=1).T.copy()  # [11, N]

    # [onehot | 1] packed per j-tile: [128, 64*11]
    yone = np.zeros((N, CK), np.float32)
    yone[np.arange(N), labels] = 1.0
    yone[:, C] = 1.0
    yone_packed = (
        yone.reshape(JT, 128, CK).transpose(1, 0, 2).reshape(128, JT * CK).copy()
    )

    # self-term kern_ii = exp(sum_c L[i,c] * A[i,c] - log_beta[i])
    kii = np.exp(
        (L.astype(np.float64) * A.astype(np.float64)).sum(axis=1)
        - log_beta.astype(np.float64)
    )
    return f, labels, apT, lpT, yone_packed, kii


def _build():
    import concourse.bacc as bacc
    import concourse.mybir as mybir
    import concourse.tile as tile

    f32 = mybir.dt.float32
    f32r = mybir.dt.float32r
    bf16 = mybir.dt.bfloat16
    kdt = bf16 if KY_BF16 else f32
    nc = bacc.Bacc(
        "TRN2",
        target_bir_lowering=False,
        debug=False,
        enable_asserts=False,
        num_devices=N_CORES,
    )
    ldt = f32r if LN_F32R else f32
    ap_d = nc.dram_tensor("apT", [CK, N], ldt, kind="ExternalInput")
    lp_d = nc.dram_tensor("lpT", [CK, LOC], ldt, kind="ExternalInput")
    yo_d = nc.dram_tensor("yone", [128, JT * CK], kdt, kind="ExternalInput")
    ky_d = nc.dram_tensor("ky", [CK, LOC], f32, kind="ExternalOutput")

    with tile.TileContext(nc) as tc:
        with (
            tc.tile_pool(name="const", bufs=1) as cp,
            tc.tile_pool(name="kern", bufs=KERN_BUFS) as kp,
            tc.tile_pool(name="ln", bufs=3, space="PSUM") as lp_pool,
            tc.tile_pool(name="kyp", bufs=1, space="PSUM") as kyp,
        ):
            ap_sb = cp.tile([CK, N], ldt)
            lp_sb = cp.tile([CK, LOC], ldt)
            yo_sb = cp.tile([128, JT * CK], kdt)
            nc.sync.dma_start(ap_sb[:], ap_d.ap())
            nc.sync.dma_start(lp_sb[:], lp_d.ap())
            nc.sync.dma_start(yo_sb[:], yo_d.ap())

            ky_ps = kyp.tile([CK, LOC], f32)

            # optional PE p-state warmup: dummy matmuls into the first ln
            # buffer (overwritten by the first real start=True matmul)
            for _ in range(WARMUP):
                wu = lp_pool.tile([128, LOC], f32, tag="ln_ps")
                nc.tensor.matmul(
                    wu[:, 0:512],
                    ap_sb[:, 0:128],
                    lp_sb[:, 0:512],
                    start=True,
                    stop=True,
                )

            # software pipeline: PE does ln(jt) before ky(jt-1) so PE never
            # stalls on ACT's exp of tile jt-1
            kern_tiles = [None] * JT
            for jt in range(JT + PIPE_DEPTH):
                if jt < JT:
                    ln_ps = lp_pool.tile([128, LOC], f32)
                    w = ap_sb[:, jt * 128 : (jt + 1) * 128]
                    for h in range(LOC // 512):
                        s = slice(h * 512, (h + 1) * 512)
                        nc.tensor.matmul(
                            ln_ps[:, s], w, lp_sb[:, s], start=True, stop=True
                        )
                    k_sb = kp.tile([128, LOC], kdt)
                    nc.scalar.activation(
                        k_sb[:], ln_ps[:], mybir.ActivationFunctionType.Exp
                    )
                    if DVE_RELEASE:
                        scr = kp.tile([1, 1], f32, tag="scr")
                        nc.vector.tensor_copy(scr[:], ln_ps[0:1, 0:1])
                    kern_tiles[jt] = k_sb
                if jt >= PIPE_DEPTH:
                    p = jt - PIPE_DEPTH
                    yw = yo_sb[:, p * CK : (p + 1) * CK]
                    kprev = kern_tiles[p]
                    if KY_FUSED:
                        nc.tensor.matmul(
                            ky_ps[:],
                            yw,
                            kprev[:],
                            start=(p == 0),
                            stop=(p == JT - 1),
                        )
                    else:
                        for h in range(LOC // 512):
                            s = slice(h * 512, (h + 1) * 512)
                            nc.tensor.matmul(
                                ky_ps[:, s],
                                yw,
                                kprev[:, s],
                                start=(p == 0),
                                stop=(p == JT - 1),
                            )
                    kern_tiles[p] = None

            if DIRECT_OUT:
                nc.sync.dma_start(ky_d.ap(), ky_ps[:])
            else:
                out_sb = cp.tile([CK, LOC], f32)
                nc.vector.tensor_copy(out_sb[:], ky_ps[:])
                nc.sync.dma_start(ky_d.ap(), out_sb[:])

    nc.compile()
    return nc, ap_d.name, lp_d.name, yo_d.name, ky_d.name


def kernel(logits, labels):
    global _compiled
    from concourse import bass_utils

    f, labels_i, apT, lpT, yone_packed, kii = _host_prep(logits, labels)

    if _compiled is None:
        _compiled = _build()
    nc, ap_name, lp_name, yo_name, ky_name = _compiled

    if KY_BF16:
        import ml_dtypes

        yone_packed = yone_packed.astype(ml_dtypes.bfloat16)
    in_maps = []
    for d in range(N_CORES):
        in_maps.append(
            {
                ap_name: apT,
                lp_name: lpT[:, d * LOC : (d + 1) * LOC].copy(),
                yo_name: yone_packed,
            }
        )
    res = bass_utils.run_bass_kernel_spmd(nc, in_maps, core_ids=list(range(N_CORES)))
    ky = np.concatenate(
        [res.results[d][ky_name] for d in range(N_CORES)], axis=1
    )  # [11, N]

    ky = ky.astype(np.float64)
    den = ky[C, :] - kii
    kern_y = ky[:C, :].T  # [N, C]
    kern_y[np.arange(N), labels_i] -= kii
    den = np.maximum(den, EPS_DEN)
    ratio = kern_y / den[:, None]
    per_sample = np.abs(ratio - f.astype(np.float64)).sum(axis=1)
    return np.asarray(per_sample.mean(), dtype=np.float32)

